# revision 2
# baseline (speedup 1.0000x reference)
"""MHLA2 Trainium2 kernel — 8-core SPMD (batch x head-group sharding).

Math (per batch b, head h):
  Q=x_q@W_Q[h], K=x_k@W_K[h], V=x_v@W_V[h]          [S, 64]
  SK = softmax(K/ds) over d (row-wise)               [S, 64]
  A  = SK^T @ V                                      [64, 64]
  Bt = softmax(Q/ds) @ A                             [S, 64]
  torch-view reshape [b,h,s,d]->[b,s',f]: head h owns output rows
  s' in [h*128,(h+1)*128); Btr_h = Bt_h.reshape(128, 1024)
  out rows = Btr_h @ W_O^T                           [128, 1024]

Sharding: core c = b*2 + g handles batch b, heads g*8..g*8+7 and writes
the contiguous output block out[b, g*1024:(g+1)*1024, :].

All DRAM traffic is bf16 (tolerance 2e-2; measured ~4e-3): inputs are
packed into two bf16 params per core (xall = [xqT; xkT; xvT], wall =
[wq | wk | wv | woT]) and the output block is bf16, halving transfer
volume over the axon tunnel, which dominates end-to-end time.

On-chip pipeline per core (S=2048, M=1024, 8 local heads):
  ph1: K-proj (xkT resident, rotated k-accum) -> exp -> rowsum ->
       normalize (f32) -> cast bf16
  ph2: V-proj per s-tile -> A accumulation (frees V tiles early)
  ph3: per f-chunk: Q-proj -> exp (unnormalized bf16, qsum via ones
       column of A_aug) -> stage5 matmul (Bt | qsum) -> normalize ->
       PE transpose -> parity-packed BtT2 (bf16) -> W_O matmuls ->
       PSUM->SBUF(bf16)->DRAM output DMA.
"""

import numpy as np
import ml_dtypes
from contextlib import ExitStack

import concourse.bass as bass
import concourse.bacc as bacc_mod
import concourse.mybir as mybir
import concourse.tile as tile
from concourse.bass_utils import run_bass_kernel_spmd
from concourse.masks import make_identity

S = 2048
M = 1024
D = 64
HL = 8            # heads per core
NK = 8            # 128-row contraction chunks of d_model
NT = 16           # 128-token tiles of S
F32 = mybir.dt.float32
BF16 = mybir.dt.bfloat16
NP_BF16 = ml_dtypes.bfloat16
AX = mybir.AxisListType
AF = mybir.ActivationFunctionType
D_SCALE = float(D) ** 0.25


def _emit(ctx, tc, nc, xall, wall, out_ext):
    xpool = ctx.enter_context(tc.tile_pool(name="x", bufs=9))
    wpool = ctx.enter_context(tc.tile_pool(name="w", bufs=8))
    wopool = ctx.enter_context(tc.tile_pool(name="wo", bufs=8))
    skpool = ctx.enter_context(tc.tile_pool(name="sk", bufs=16))
    skfpool = ctx.enter_context(tc.tile_pool(name="skf", bufs=2))
    vpool = ctx.enter_context(tc.tile_pool(name="v", bufs=3))
    qpool = ctx.enter_context(tc.tile_pool(name="qT", bufs=2))
    btpool = ctx.enter_context(tc.tile_pool(name="bt", bufs=2))
    spool = ctx.enter_context(tc.tile_pool(name="small", bufs=36))
    bnpool = ctx.enter_context(tc.tile_pool(name="bn", bufs=4))
    opool = ctx.enter_context(tc.tile_pool(name="osb", bufs=2))
    cpool = ctx.enter_context(tc.tile_pool(name="const", bufs=2))
    ppool = ctx.enter_context(tc.tile_pool(name="pbig", bufs=3, space="PSUM"))
    papool = ctx.enter_context(tc.tile_pool(name="pa", bufs=1, space="PSUM"))
    p5pool = ctx.enter_context(tc.tile_pool(name="p5", bufs=2, space="PSUM"))
    ptpool = ctx.enter_context(tc.tile_pool(name="pt", bufs=2, space="PSUM"))

    ident = cpool.tile([128, 128], F32)
    make_identity(nc, ident[:])

    def load_x(pool, row_base, tag):
        tiles = []
        for k in range(NK):
            t = pool.tile([128, S], BF16, tag=tag)
            r = row_base + k * 128
            nc.gpsimd.dma_start(out=t[:], in_=xall[r:r + 128, :])
            tiles.append(t)
        return tiles

    def load_w(pool, col_lo, width, tag):
        tiles = []
        for k in range(NK):
            t = pool.tile([128, width], BF16, tag=tag)
            nc.gpsimd.dma_start(
                out=t[:],
                in_=wall[k * 128:(k + 1) * 128, col_lo:col_lo + width],
            )
            tiles.append(t)
        return tiles

    # ---------------- phase 1: K projection + softmax ----------------
    xk_sb = load_x(xpool, M, "x")
    wk_sb = load_w(wpool, 512, 512, "w")

    sk_sb = []
    for t in range(NT):
        ps = ppool.tile([128, 512], F32, tag="pbig")
        for j in range(NK):
            k = (t + j) % NK
            nc.tensor.matmul(
                ps[:],
                xk_sb[k][:, t * 128:(t + 1) * 128],
                wk_sb[k][:],
                start=(j == 0),
                stop=(j == NK - 1),
            )
        skf = skfpool.tile([128, 512], F32, tag="skf")
        nc.scalar.activation(skf[:], ps[:], AF.Exp)
        ksum = spool.tile([128, 8], F32, tag="ksum")
        nc.vector.reduce_sum(
            ksum[:], skf[:].rearrange("p (h d) -> p h d", d=D), axis=AX.X
        )
        krec = spool.tile([128, 8], F32, tag="krec")
        nc.vector.reciprocal(krec[:], ksum[:])
        for h in range(HL):
            nc.vector.tensor_scalar_mul(
                skf[:, h * D:(h + 1) * D], skf[:, h * D:(h + 1) * D],
                krec[:, h:h + 1],
            )
        sk = skpool.tile([128, 512], BF16, tag="sk")
        nc.vector.tensor_copy(sk[:], skf[:])
        sk_sb.append(sk)

    # ---------------- phase 2: V projection + A accumulation ----------------
    xv_sb = load_x(xpool, 2 * M, "x")
    wv_sb = load_w(wpool, 1024, 512, "w")
    wo_sb = load_w(wopool, 1536, M, "wo")

    pa = papool.tile([64, 512], F32, tag="pa")
    for t in range(NT):
        ps = ppool.tile([128, 512], F32, tag="pbig")
        for j in range(NK):
            k = (t + j) % NK
            nc.tensor.matmul(
                ps[:],
                xv_sb[k][:, t * 128:(t + 1) * 128],
                wv_sb[k][:],
                start=(j == 0),
                stop=(j == NK - 1),
            )
        vt = vpool.tile([128, 512], BF16, tag="v")
        nc.scalar.copy(vt[:], ps[:])
        for h in range(HL):
            # One accumulation group for the whole bank: start clears the
            # entire PSUM bank, so only the very first matmul may set it.
            nc.tensor.matmul(
                pa[:, h * D:(h + 1) * D],
                sk_sb[t][:, h * D:(h + 1) * D],
                vt[:, h * D:(h + 1) * D],
                start=(t == 0 and h == 0),
                stop=(t == NT - 1 and h == HL - 1),
                skip_group_check=True,
            )

    # A_aug: per head [64, 65] = [A_h | ones]; stride-65 packing.
    # Rows 64-127 hold a copy so stage5 rhs base_partition can match the
    # lhsT slice (qt rows 64-127 for odd local heads).
    a_aug = cpool.tile([128, HL * 65], BF16)
    nc.gpsimd.memset(
        a_aug[0:64, :].rearrange("p (h c) -> p h c", c=65)[:, :, 64:65], 1.0
    )
    nc.vector.tensor_copy(
        a_aug[0:64, :].rearrange("p (h c) -> p h c", c=65)[:, :, 0:64],
        pa[:].rearrange("p (h d) -> p h d", d=D),
    )
    nc.sync.dma_start(out=a_aug[64:128, :], in_=a_aug[0:64, :])

    # ---------------- phase 3: Q -> expQ^T -> Bt -> W_O ----------------
    xq_sb = load_x(xpool, 0, "x")
    wq_sb = load_w(wpool, 0, 512, "w")

    for fc in range(4):
        qt = qpool.tile([128, S], BF16, tag="qT")
        for sc in range(4):
            ps = ppool.tile([128, 512], F32, tag="pbig")
            for j in range(NK):
                k = (sc + j) % NK
                nc.tensor.matmul(
                    ps[:],
                    wq_sb[k][:, fc * 128:(fc + 1) * 128],
                    xq_sb[k][:, sc * 512:(sc + 1) * 512],
                    start=(j == 0),
                    stop=(j == NK - 1),
                )
            nc.scalar.activation(qt[:, sc * 512:(sc + 1) * 512], ps[:], AF.Exp)

        for hh in range(2):
            h = 2 * fc + hh       # local head
            bt2 = btpool.tile([128, M], BF16, tag="bt")
            for t in range(NT):
                p5 = p5pool.tile([128, 65], F32, tag="p5")
                nc.tensor.matmul(
                    p5[:],
                    qt[hh * 64:(hh + 1) * 64, t * 128:(t + 1) * 128],
                    a_aug[hh * 64:(hh + 1) * 64, h * 65:(h + 1) * 65],
                    start=True,
                    stop=True,
                )
                qrec = spool.tile([128, 1], F32, tag="qrec")
                nc.vector.reciprocal(qrec[:], p5[:, 64:65])
                bn = bnpool.tile([128, 64], F32, tag="bn")
                nc.vector.tensor_scalar_mul(bn[:], p5[:, 0:64], qrec[:])
                pt = ptpool.tile([64, 128], F32, tag="pt")
                nc.tensor.transpose(
                    pt[:], bn[:],
                    ident[:],
                )
                ptv = pt[:].rearrange("p (q two) -> p two q", two=2)
                if t % 2 == 0:
                    nc.scalar.copy(bt2[0:64, t * 64:(t + 1) * 64], ptv[:, 0, :])
                    nc.vector.tensor_copy(
                        bt2[64:128, t * 64:(t + 1) * 64], ptv[:, 1, :]
                    )
                else:
                    nc.vector.tensor_copy(
                        bt2[0:64, t * 64:(t + 1) * 64], ptv[:, 0, :]
                    )
                    nc.scalar.copy(bt2[64:128, t * 64:(t + 1) * 64], ptv[:, 1, :])

            bt2v = bt2[:].rearrange("p (q c) -> p c q", c=8)
            for oh in range(2):
                po = ppool.tile([128, 512], F32, tag="pbig")
                for c in range(NK):
                    nc.tensor.matmul(
                        po[:],
                        bt2v[:, c, :],
                        wo_sb[c][:, oh * 512:(oh + 1) * 512],
                        start=(c == 0),
                        stop=(c == NK - 1),
                    )
                ob = opool.tile([128, 512], BF16, tag="osb")
                nc.scalar.copy(ob[:], po[:])
                nc.sync.dma_start(
                    out=out_ext[h * 128:(h + 1) * 128, oh * 512:(oh + 1) * 512],
                    in_=ob[:],
                )


_NC_CACHE = None


def _build():
    global _NC_CACHE
    if _NC_CACHE is not None:
        return _NC_CACHE
    nc = bacc_mod.Bacc(None, target_bir_lowering=False)
    xall = nc.declare_dram_parameter("xall", [3 * M, S], BF16, isOutput=False)
    wall = nc.declare_dram_parameter("wall", [M, 2560], BF16, isOutput=False)
    out = nc.declare_dram_parameter("out", [HL * 128, M], BF16, isOutput=True)
    with tile.TileContext(nc) as tc, ExitStack() as ctx:
        _emit(ctx, tc, nc, xall, wall, out)
    if not nc.is_finalized():
        nc.finalize()
    _NC_CACHE = nc
    return nc


def _in_maps(x_q, x_k, x_v, W_Q, W_K, W_V, W_O):
    xalls = []
    for b in range(4):
        xa = np.empty((3 * M, S), dtype=NP_BF16)
        xa[0:M] = x_q[b].T
        xa[M:2 * M] = x_k[b].T
        xa[2 * M:3 * M] = x_v[b].T
        xalls.append(xa)
    walls = []
    for g in range(2):
        sl = slice(g * HL, (g + 1) * HL)
        wa = np.empty((M, 2560), dtype=NP_BF16)
        wa[:, 0:512] = (W_Q[sl] / D_SCALE).transpose(1, 0, 2).reshape(M, 512)
        wa[:, 512:1024] = (W_K[sl] / D_SCALE).transpose(1, 0, 2).reshape(M, 512)
        wa[:, 1024:1536] = W_V[sl].transpose(1, 0, 2).reshape(M, 512)
        wa[:, 1536:2560] = W_O.T
        walls.append(wa)
    return [{"xall": xalls[b], "wall": walls[g]}
            for b in range(4) for g in range(2)]


def run(inputs, **kw):
    nc = _build()
    maps = _in_maps(inputs["x_q"], inputs["x_k"], inputs["x_v"],
                    inputs["W_Q"], inputs["W_K"], inputs["W_V"],
                    inputs["W_O"])
    res = run_bass_kernel_spmd(nc, maps, list(range(8)), **kw)
    out = np.empty((4, S, M), dtype=np.float32)
    for b in range(4):
        for g in range(2):
            out[b, g * M:(g + 1) * M, :] = res.results[b * 2 + g]["out"]
    return out, res


def kernel(**inputs):
    out, _ = run(inputs)
    return out


# revision 10
# speedup vs baseline: 72.6338x; 72.6338x over previous
"""MHLA2 Trainium2 kernel — 8-core SPMD (batch x head-group sharding).

Math (per batch b, head h):
  Q=x_q@W_Q[h], K=x_k@W_K[h], V=x_v@W_V[h]          [S, 64]
  SK = softmax(K/ds) over d (row-wise)               [S, 64]
  A  = SK^T @ V                                      [64, 64]
  Bt = softmax(Q/ds) @ A                             [S, 64]
  torch-view reshape [b,h,s,d]->[b,s',f]: head h owns output rows
  s' in [h*128,(h+1)*128); Btr_h = Bt_h.reshape(128, 1024)
  out rows = Btr_h @ W_O^T                           [128, 1024]

Sharding: core c = b*2 + g handles batch b, heads g*8..g*8+7 and writes
the contiguous output block out[b, g*1024:(g+1)*1024, :].

All DRAM traffic is bf16 (tolerance 2e-2; measured ~4e-3): inputs are
packed into two bf16 params per core and the output block is bf16.
Host->device transfer over the axon tunnel dominates end-to-end time,
so each core uploads only a unique shard and the duplicated full
tensors are reconstructed on-chip with AllGather collectives:
  xsh [1536, 2048] = half of xall = [xqT; xkT; xvT] of this core's
      batch; pairwise AllGather [[0,1],[2,3],[4,5],[6,7]] rebuilds the
      full xall (both cores of a batch need the same x).
  wsh [256, 2560] = quarter of wall = [wq | wk | wv | woT] of this
      core's head-group; AllGather across [[0,2,4,6],[1,3,5,7]] (the
      cores sharing a head-group) rebuilds the full wall.

On-chip pipeline per core (S=2048, M=1024, 8 local heads):
  ph1: K-proj (xkT resident, rotated k-accum) -> exp -> rowsum ->
       normalize (f32) -> cast bf16
  ph2: V-proj per s-tile -> A accumulation (frees V tiles early)
  ph3: per f-chunk: Q-proj -> exp (unnormalized bf16, qsum via ones
       column of A_aug) -> stage5 matmul (Bt | qsum) -> normalize ->
       PE transpose -> parity-packed BtT2 (bf16) -> W_O matmuls ->
       PSUM->SBUF(bf16)->DRAM output DMA.
"""

import numpy as np
import ml_dtypes
from contextlib import ExitStack

import jax

# Persist XLA-compiled executables across processes: run_bass_kernel_spmd
# re-jits a fresh closure every call, so without this each call pays
# ~0.25-0.45s re-compiling the identical wrapper module (the inner NEFF is
# already disk-cached by neuronxcc separately).
try:
    jax.config.update("jax_compilation_cache_dir", "/tmp/.jax_comp_cache")
    jax.config.update("jax_persistent_cache_min_entry_size_bytes", -1)
    jax.config.update("jax_persistent_cache_min_compile_time_secs", 0)
except Exception:
    pass

import concourse.bacc as bacc_mod
import concourse.mybir as mybir
import concourse.tile as tile
from concourse.bass_utils import run_bass_kernel_spmd
from concourse.masks import make_identity

S = 2048
M = 1024
D = 64
HL = 8            # heads per core
NK = 8            # 128-row contraction chunks of d_model
NT = 16           # 128-token tiles of S
F32 = mybir.dt.float32
BF16 = mybir.dt.bfloat16
NP_BF16 = ml_dtypes.bfloat16
AX = mybir.AxisListType
AF = mybir.ActivationFunctionType
D_SCALE = float(D) ** 0.25


def _emit(ctx, tc, nc, xsh, wsh, out_ext):
    dpool = ctx.enter_context(tc.tile_pool(name="dram", bufs=1, space="DRAM"))
    xin = dpool.tile([3 * M // 2, S], BF16)
    xall = dpool.tile([3 * M, S], BF16)
    win = dpool.tile([256, 2560], BF16)
    wall = dpool.tile([M, 2560], BF16)
    nc.gpsimd.dma_start(win[:], wsh[:])
    nc.gpsimd.dma_start(xin[:], xsh[:])
    nc.gpsimd.collective_compute(
        "AllGather", mybir.AluOpType.bypass,
        replica_groups=[[0, 2, 4, 6], [1, 3, 5, 7]],
        ins=[win.opt()], outs=[wall.opt()],
    )
    nc.gpsimd.collective_compute(
        "AllGather", mybir.AluOpType.bypass,
        replica_groups=[[0, 1], [2, 3], [4, 5], [6, 7]],
        ins=[xin.opt()], outs=[xall.opt()],
    )

    xpool = ctx.enter_context(tc.tile_pool(name="x", bufs=9))
    wpool = ctx.enter_context(tc.tile_pool(name="w", bufs=8))
    wopool = ctx.enter_context(tc.tile_pool(name="wo", bufs=8))
    skpool = ctx.enter_context(tc.tile_pool(name="sk", bufs=16))
    skfpool = ctx.enter_context(tc.tile_pool(name="skf", bufs=2))
    vpool = ctx.enter_context(tc.tile_pool(name="v", bufs=3))
    qpool = ctx.enter_context(tc.tile_pool(name="qT", bufs=2))
    btpool = ctx.enter_context(tc.tile_pool(name="bt", bufs=2))
    spool = ctx.enter_context(tc.tile_pool(name="small", bufs=36))
    bnpool = ctx.enter_context(tc.tile_pool(name="bn", bufs=4))
    opool = ctx.enter_context(tc.tile_pool(name="osb", bufs=2))
    cpool = ctx.enter_context(tc.tile_pool(name="const", bufs=2))
    ppool = ctx.enter_context(tc.tile_pool(name="pbig", bufs=3, space="PSUM"))
    papool = ctx.enter_context(tc.tile_pool(name="pa", bufs=1, space="PSUM"))
    p5pool = ctx.enter_context(tc.tile_pool(name="p5", bufs=2, space="PSUM"))
    ptpool = ctx.enter_context(tc.tile_pool(name="pt", bufs=2, space="PSUM"))

    ident = cpool.tile([128, 128], F32)
    make_identity(nc, ident[:])

    def load_x(pool, row_base, tag):
        tiles = []
        for k in range(NK):
            t = pool.tile([128, S], BF16, tag=tag)
            r = row_base + k * 128
            nc.gpsimd.dma_start(out=t[:], in_=xall[r:r + 128, :])
            tiles.append(t)
        return tiles

    def load_w(pool, col_lo, width, tag):
        tiles = []
        for k in range(NK):
            t = pool.tile([128, width], BF16, tag=tag)
            nc.gpsimd.dma_start(
                out=t[:],
                in_=wall[k * 128:(k + 1) * 128, col_lo:col_lo + width],
            )
            tiles.append(t)
        return tiles

    # ---------------- phase 1: K projection + softmax ----------------
    xk_sb = load_x(xpool, M, "x")
    wk_sb = load_w(wpool, 512, 512, "w")

    sk_sb = []
    for t in range(NT):
        ps = ppool.tile([128, 512], F32, tag="pbig")
        for j in range(NK):
            k = (t + j) % NK
            nc.tensor.matmul(
                ps[:],
                xk_sb[k][:, t * 128:(t + 1) * 128],
                wk_sb[k][:],
                start=(j == 0),
                stop=(j == NK - 1),
            )
        skf = skfpool.tile([128, 512], F32, tag="skf")
        nc.scalar.activation(skf[:], ps[:], AF.Exp)
        ksum = spool.tile([128, 8], F32, tag="ksum")
        nc.vector.reduce_sum(
            ksum[:], skf[:].rearrange("p (h d) -> p h d", d=D), axis=AX.X
        )
        krec = spool.tile([128, 8], F32, tag="krec")
        nc.vector.reciprocal(krec[:], ksum[:])
        for h in range(HL):
            nc.vector.tensor_scalar_mul(
                skf[:, h * D:(h + 1) * D], skf[:, h * D:(h + 1) * D],
                krec[:, h:h + 1],
            )
        sk = skpool.tile([128, 512], BF16, tag="sk")
        nc.vector.tensor_copy(sk[:], skf[:])
        sk_sb.append(sk)

    # ---------------- phase 2: V projection + A accumulation ----------------
    xv_sb = load_x(xpool, 2 * M, "x")
    wv_sb = load_w(wpool, 1024, 512, "w")
    wo_sb = load_w(wopool, 1536, M, "wo")

    pa = papool.tile([64, 512], F32, tag="pa")
    for t in range(NT):
        ps = ppool.tile([128, 512], F32, tag="pbig")
        for j in range(NK):
            k = (t + j) % NK
            nc.tensor.matmul(
                ps[:],
                xv_sb[k][:, t * 128:(t + 1) * 128],
                wv_sb[k][:],
                start=(j == 0),
                stop=(j == NK - 1),
            )
        vt = vpool.tile([128, 512], BF16, tag="v")
        nc.scalar.copy(vt[:], ps[:])
        for h in range(HL):
            # One accumulation group for the whole bank: start clears the
            # entire PSUM bank, so only the very first matmul may set it.
            nc.tensor.matmul(
                pa[:, h * D:(h + 1) * D],
                sk_sb[t][:, h * D:(h + 1) * D],
                vt[:, h * D:(h + 1) * D],
                start=(t == 0 and h == 0),
                stop=(t == NT - 1 and h == HL - 1),
                skip_group_check=True,
            )

    # A_aug: per head [64, 65] = [A_h | ones]; stride-65 packing.
    # Rows 64-127 hold a copy so stage5 rhs base_partition can match the
    # lhsT slice (qt rows 64-127 for odd local heads).
    a_aug = cpool.tile([128, HL * 65], BF16)
    nc.gpsimd.memset(
        a_aug[0:64, :].rearrange("p (h c) -> p h c", c=65)[:, :, 64:65], 1.0
    )
    nc.vector.tensor_copy(
        a_aug[0:64, :].rearrange("p (h c) -> p h c", c=65)[:, :, 0:64],
        pa[:].rearrange("p (h d) -> p h d", d=D),
    )
    nc.sync.dma_start(out=a_aug[64:128, :], in_=a_aug[0:64, :])

    # ---------------- phase 3: Q -> expQ^T -> Bt -> W_O ----------------
    xq_sb = load_x(xpool, 0, "x")
    wq_sb = load_w(wpool, 0, 512, "w")

    for fc in range(4):
        qt = qpool.tile([128, S], BF16, tag="qT")
        for sc in range(4):
            ps = ppool.tile([128, 512], F32, tag="pbig")
            for j in range(NK):
                k = (sc + j) % NK
                nc.tensor.matmul(
                    ps[:],
                    wq_sb[k][:, fc * 128:(fc + 1) * 128],
                    xq_sb[k][:, sc * 512:(sc + 1) * 512],
                    start=(j == 0),
                    stop=(j == NK - 1),
                )
            nc.scalar.activation(qt[:, sc * 512:(sc + 1) * 512], ps[:], AF.Exp)

        for hh in range(2):
            h = 2 * fc + hh       # local head
            bt2 = btpool.tile([128, M], BF16, tag="bt")
            for t in range(NT):
                p5 = p5pool.tile([128, 65], F32, tag="p5")
                nc.tensor.matmul(
                    p5[:],
                    qt[hh * 64:(hh + 1) * 64, t * 128:(t + 1) * 128],
                    a_aug[hh * 64:(hh + 1) * 64, h * 65:(h + 1) * 65],
                    start=True,
                    stop=True,
                )
                qrec = spool.tile([128, 1], F32, tag="qrec")
                nc.vector.reciprocal(qrec[:], p5[:, 64:65])
                bn = bnpool.tile([128, 64], F32, tag="bn")
                nc.vector.tensor_scalar_mul(bn[:], p5[:, 0:64], qrec[:])
                pt = ptpool.tile([64, 128], F32, tag="pt")
                nc.tensor.transpose(
                    pt[:], bn[:],
                    ident[:],
                )
                ptv = pt[:].rearrange("p (q two) -> p two q", two=2)
                if t % 2 == 0:
                    nc.scalar.copy(bt2[0:64, t * 64:(t + 1) * 64], ptv[:, 0, :])
                    nc.vector.tensor_copy(
                        bt2[64:128, t * 64:(t + 1) * 64], ptv[:, 1, :]
                    )
                else:
                    nc.vector.tensor_copy(
                        bt2[0:64, t * 64:(t + 1) * 64], ptv[:, 0, :]
                    )
                    nc.scalar.copy(bt2[64:128, t * 64:(t + 1) * 64], ptv[:, 1, :])

            bt2v = bt2[:].rearrange("p (q c) -> p c q", c=8)
            for oh in range(2):
                po = ppool.tile([128, 512], F32, tag="pbig")
                for c in range(NK):
                    nc.tensor.matmul(
                        po[:],
                        bt2v[:, c, :],
                        wo_sb[c][:, oh * 512:(oh + 1) * 512],
                        start=(c == 0),
                        stop=(c == NK - 1),
                    )
                ob = opool.tile([128, 512], BF16, tag="osb")
                nc.scalar.copy(ob[:], po[:])
                nc.sync.dma_start(
                    out=out_ext[h * 128:(h + 1) * 128, oh * 512:(oh + 1) * 512],
                    in_=ob[:],
                )


_NC_CACHE = None


def _build():
    global _NC_CACHE
    if _NC_CACHE is not None:
        return _NC_CACHE
    nc = bacc_mod.Bacc(None, target_bir_lowering=False, num_devices=8)
    xsh = nc.declare_dram_parameter("xsh", [3 * M // 2, S], BF16,
                                    isOutput=False)
    wsh = nc.declare_dram_parameter("wsh", [256, 2560], BF16, isOutput=False)
    out = nc.declare_dram_parameter("out", [HL * 128, M], BF16, isOutput=True)
    with tile.TileContext(nc) as tc, ExitStack() as ctx:
        _emit(ctx, tc, nc, xsh, wsh, out)
    if not nc.is_finalized():
        nc.finalize()
    _NC_CACHE = nc
    return nc


def _in_maps(x_q, x_k, x_v, W_Q, W_K, W_V, W_O):
    xalls = []
    for b in range(4):
        xa = np.empty((3 * M, S), dtype=NP_BF16)
        # cast to bf16 while still contiguous (fast), then transpose the
        # 2-byte elements — ~2x faster than a strided f32->bf16 assign
        xa[0:M] = x_q[b].astype(NP_BF16).T
        xa[M:2 * M] = x_k[b].astype(NP_BF16).T
        xa[2 * M:3 * M] = x_v[b].astype(NP_BF16).T
        xalls.append(xa)
    walls = []
    for g in range(2):
        sl = slice(g * HL, (g + 1) * HL)
        wa = np.empty((M, 2560), dtype=NP_BF16)
        wa[:, 0:512] = (W_Q[sl] / D_SCALE).transpose(1, 0, 2).reshape(M, 512)
        wa[:, 512:1024] = (W_K[sl] / D_SCALE).transpose(1, 0, 2).reshape(M, 512)
        wa[:, 1024:1536] = W_V[sl].transpose(1, 0, 2).reshape(M, 512)
        wa[:, 1536:2560] = W_O.T
        walls.append(wa)
    # Shards only: core (b, g) uploads half of its batch's xall and a
    # quarter of its head-group's wall; AllGathers on-chip rebuild the
    # full tensors (see module docstring).  Views, no host copies.
    half = 3 * M // 2
    return [{"xsh": xalls[b][g * half:(g + 1) * half],
             "wsh": walls[g][b * 256:(b + 1) * 256]}
            for b in range(4) for g in range(2)]


def run(inputs, **kw):
    nc = _build()
    maps = _in_maps(inputs["x_q"], inputs["x_k"], inputs["x_v"],
                    inputs["W_Q"], inputs["W_K"], inputs["W_V"],
                    inputs["W_O"])
    res = run_bass_kernel_spmd(nc, maps, list(range(8)), **kw)
    out = np.empty((4, S, M), dtype=np.float32)
    for b in range(4):
        for g in range(2):
            out[b, g * M:(g + 1) * M, :] = res.results[b * 2 + g]["out"]
    return out, res


def kernel(**inputs):
    out, _ = run(inputs)
    return out


def measure_exec_ns(inputs, iters=3):
    """Tightest measurable upper bound on NEFF execution time.

    NTFF profiling (run_bass_kernel_spmd(trace=True)) is unavailable under
    this axon client (no antenv.axon_hooks), so time the blocking execute of
    the same compiled program with all inputs already staged on the devices:
    excludes host prep and H2D/D2H of a normal call, still includes the
    dispatch round-trip on top of on-silicon time.  Returns min-of-iters in
    ns, or None if the measurement path is unavailable.
    """
    import time
    try:
        import jax
        from jax.sharding import Mesh, PartitionSpec, NamedSharding
        from jax.experimental.shard_map import shard_map
        from concourse.bass2jax import (
            _bass_exec_p, partition_id_tensor, install_neuronx_cc_hook,
        )

        nc = _build()
        maps = _in_maps(inputs["x_q"], inputs["x_k"], inputs["x_v"],
                        inputs["W_Q"], inputs["W_K"], inputs["W_V"],
                        inputs["W_O"])
        install_neuronx_cc_hook()

        partition_name = (nc.partition_id_tensor.name
                          if nc.partition_id_tensor else None)
        in_names, out_names, out_avals, zero_outs = [], [], [], []
        for alloc in nc.m.functions[0].allocations:
            if not isinstance(alloc, mybir.MemoryLocationSet):
                continue
            name = alloc.memorylocations[0].name
            if alloc.kind == "ExternalInput":
                if name != partition_name:
                    in_names.append(name)
            elif alloc.kind == "ExternalOutput":
                out_names.append(name)
                shape = tuple(alloc.tensor_shape)
                dtype = mybir.dt.np(alloc.dtype)
                out_avals.append(jax.core.ShapedArray(shape, dtype))
                zero_outs.append(np.zeros(shape, dtype))
        n_params = len(in_names)
        n_outs = len(out_avals)
        in_names.extend(out_names)
        if partition_name is not None:
            in_names.append(partition_name)
        donate = tuple(range(n_params, n_params + n_outs))

        def _body(*args):
            operands = list(args)
            if partition_name is not None:
                operands.append(partition_id_tensor())
            outs = _bass_exec_p.bind(
                *operands, out_avals=tuple(out_avals),
                in_names=tuple(in_names), out_names=tuple(out_names),
                lowering_input_output_aliases=(), sim_require_finite=True,
                sim_require_nnan=True, nc=nc)
            return tuple(outs)

        n_cores = 8
        devices = jax.devices()[:n_cores]
        mesh = Mesh(np.asarray(devices), ("core",))
        in_specs = (PartitionSpec("core"),) * (n_params + n_outs)
        out_specs = (PartitionSpec("core"),) * len(out_names)
        sharded = jax.jit(
            shard_map(_body, mesh=mesh, in_specs=in_specs,
                      out_specs=out_specs, check_rep=False),
            donate_argnums=donate, keep_unused=True)

        per_core = [[np.asarray(m[name]) for name in in_names[:n_params]]
                    for m in maps]
        concat_in = [
            np.concatenate([per_core[c][i] for c in range(n_cores)], axis=0)
            for i in range(n_params)
        ]
        concat_zeros = [
            np.zeros((n_cores * z.shape[0], *z.shape[1:]), z.dtype)
            for z in zero_outs
        ]
        compiled = sharded.lower(*concat_in, *concat_zeros).compile()
        sharding = NamedSharding(mesh, PartitionSpec("core"))
        dev_in = [jax.device_put(a, sharding) for a in concat_in]
        for d in dev_in:
            d.block_until_ready()

        best = None
        for _ in range(iters):
            # output buffers are donated, so re-stage them outside the
            # timed region each iteration
            dev_zero = [jax.device_put(a, sharding) for a in concat_zeros]
            for d in dev_zero:
                d.block_until_ready()
            t0 = time.time()
            outs = compiled(*dev_in, *dev_zero)
            for o in outs:
                o.block_until_ready()
            dt = time.time() - t0
            if best is None or dt < best:
                best = dt
        return int(best * 1e9)
    except Exception:
        return None


# revision 14
# speedup vs baseline: 14350.4021x; 197.5720x over previous
"""MHLA2 Trainium2 kernel — 8-core SPMD (batch x head-group sharding).

Math (per batch b, head h):
  Q=x_q@W_Q[h], K=x_k@W_K[h], V=x_v@W_V[h]          [S, 64]
  SK = softmax(K/ds) over d (row-wise)               [S, 64]
  A  = SK^T @ V                                      [64, 64]
  Bt = softmax(Q/ds) @ A                             [S, 64]
  torch-view reshape [b,h,s,d]->[b,s',f]: head h owns output rows
  s' in [h*128,(h+1)*128); Btr_h = Bt_h.reshape(128, 1024)
  out rows = Btr_h @ W_O^T                           [128, 1024]

Sharding: core c = b*2 + g handles batch b, heads g*8..g*8+7 and writes
the contiguous output block out[b, g*1024:(g+1)*1024, :].

All DRAM traffic is bf16 (tolerance 2e-2; measured ~4e-3): inputs are
packed into two bf16 params per core (xall = [xqT; xkT; xvT], wall =
[wq | wk | wv | woT]) and the output block is bf16.  Inputs are
uploaded directly (no collectives): NTFF profiling showed the
AllGather path costs ~300us of serial CC-stream time (init barrier
56us + ~45us fixed per gather) against a ~250us compute span, so
on-silicon exec time is lower with plain duplicated uploads.

On-chip pipeline per core (S=2048, M=1024, 8 local heads):
  ph1: K-proj (xkT resident, rotated k-accum) -> exp -> rowsum ->
       normalize (f32) -> cast bf16
  ph2: V-proj per s-tile -> A accumulation (frees V tiles early) ->
       A_aug = per-head [A_h | ones] with rows 64-127 duplicated
  ph3: per f-chunk: Q-proj -> exp (unnormalized bf16) -> per head:
       BtT_unnorm = A_aug_h^T @ expQ^T in one N=512 matmul per s-chunk
       ([65, 512] PSUM: rows 0-63 = Bt^T, row 64 = qsum), qsum row
       broadcast to 64 partitions via a rank-1 matmul, reciprocal +
       multiply on DVE -> btT (bf16, [64, S]); rows 64-127 get a copy
       shifted left one column so the W_O lhsT [(tok,d), r] chunks are
       single strided views btTv[:, 2c, :]; W_O matmuls -> bf16 out.
       (This replaces a per-s-tile PE-transpose + normalize + parity
       copy chain that serialized DVE/ACT and let HAM re-throttle.)
"""

import numpy as np
import ml_dtypes
from contextlib import ExitStack

import jax

# Persist XLA-compiled executables across processes: run_bass_kernel_spmd
# re-jits a fresh closure every call, so without this each call pays
# ~0.25-0.45s re-compiling the identical wrapper module (the inner NEFF is
# already disk-cached by neuronxcc separately).
try:
    jax.config.update("jax_compilation_cache_dir", "/tmp/.jax_comp_cache")
    jax.config.update("jax_persistent_cache_min_entry_size_bytes", -1)
    jax.config.update("jax_persistent_cache_min_compile_time_secs", 0)
except Exception:
    pass

import concourse.bacc as bacc_mod
import concourse.mybir as mybir
import concourse.tile as tile
from concourse.bass_utils import run_bass_kernel_spmd

S = 2048
M = 1024
D = 64
HL = 8            # heads per core
NK = 8            # 128-row contraction chunks of d_model
NT = 16           # 128-token tiles of S
F32 = mybir.dt.float32
F32R = mybir.dt.float32r
BF16 = mybir.dt.bfloat16
NP_BF16 = ml_dtypes.bfloat16
AX = mybir.AxisListType
AF = mybir.ActivationFunctionType
D_SCALE = float(D) ** 0.25


def _emit(ctx, tc, nc, xall, wall, out_ext):
    xpool = ctx.enter_context(tc.tile_pool(name="x", bufs=9))
    wpool = ctx.enter_context(tc.tile_pool(name="w", bufs=8))
    wopool = ctx.enter_context(tc.tile_pool(name="wo", bufs=8))
    skpool = ctx.enter_context(tc.tile_pool(name="sk", bufs=16))
    skfpool = ctx.enter_context(tc.tile_pool(name="skf", bufs=2))
    vpool = ctx.enter_context(tc.tile_pool(name="v", bufs=3))
    qpool = ctx.enter_context(tc.tile_pool(name="qT", bufs=2))
    btpool = ctx.enter_context(tc.tile_pool(name="btT", bufs=2))
    spool = ctx.enter_context(tc.tile_pool(name="small", bufs=8))
    qsrpool = ctx.enter_context(tc.tile_pool(name="qsr", bufs=4))
    recpool = ctx.enter_context(tc.tile_pool(name="rec", bufs=4))
    opool = ctx.enter_context(tc.tile_pool(name="osb", bufs=2))
    cpool = ctx.enter_context(tc.tile_pool(name="const", bufs=2))
    ppool = ctx.enter_context(tc.tile_pool(name="pbig", bufs=3, space="PSUM"))
    papool = ctx.enter_context(tc.tile_pool(name="pa", bufs=1, space="PSUM"))
    p6pool = ctx.enter_context(tc.tile_pool(name="p6", bufs=2, space="PSUM"))
    pbpool = ctx.enter_context(tc.tile_pool(name="pb", bufs=2, space="PSUM"))

    ones_col = cpool.tile([1, 64], F32)
    nc.gpsimd.memset(ones_col[:], 1.0)

    def load_x(pool, row_base, tag):
        tiles = []
        for k in range(NK):
            t = pool.tile([128, S], BF16, tag=tag)
            r = row_base + k * 128
            nc.gpsimd.dma_start(out=t[:], in_=xall[r:r + 128, :])
            tiles.append(t)
        return tiles

    def load_w(pool, col_lo, width, tag):
        tiles = []
        for k in range(NK):
            t = pool.tile([128, width], BF16, tag=tag)
            nc.gpsimd.dma_start(
                out=t[:],
                in_=wall[k * 128:(k + 1) * 128, col_lo:col_lo + width],
            )
            tiles.append(t)
        return tiles

    # ---------------- phase 1: K projection + softmax ----------------
    xk_sb = load_x(xpool, M, "x")
    wk_sb = load_w(wpool, 512, 512, "w")

    sk_sb = []
    for t in range(NT):
        ps = ppool.tile([128, 512], F32, tag="pbig")
        for j in range(NK):
            k = (t + j) % NK
            nc.tensor.matmul(
                ps[:],
                xk_sb[k][:, t * 128:(t + 1) * 128],
                wk_sb[k][:],
                start=(j == 0),
                stop=(j == NK - 1),
            )
        skf = skfpool.tile([128, 512], F32, tag="skf")
        nc.scalar.activation(skf[:], ps[:], AF.Exp)
        ksum = spool.tile([128, 8], F32, tag="ksum")
        nc.vector.reduce_sum(
            ksum[:], skf[:].rearrange("p (h d) -> p h d", d=D), axis=AX.X
        )
        krec = spool.tile([128, 8], F32, tag="krec")
        nc.vector.reciprocal(krec[:], ksum[:])
        for h in range(HL):
            nc.vector.tensor_scalar_mul(
                skf[:, h * D:(h + 1) * D], skf[:, h * D:(h + 1) * D],
                krec[:, h:h + 1],
            )
        sk = skpool.tile([128, 512], BF16, tag="sk")
        nc.scalar.copy(sk[:], skf[:])
        sk_sb.append(sk)

    # ---------------- phase 2: V projection + A accumulation ----------------
    xv_sb = load_x(xpool, 2 * M, "x")
    wv_sb = load_w(wpool, 1024, 512, "w")
    wo_sb = load_w(wopool, 1536, M, "wo")

    pa = papool.tile([64, 512], F32, tag="pa")
    for t in range(NT):
        ps = ppool.tile([128, 512], F32, tag="pbig")
        for j in range(NK):
            k = (t + j) % NK
            nc.tensor.matmul(
                ps[:],
                xv_sb[k][:, t * 128:(t + 1) * 128],
                wv_sb[k][:],
                start=(j == 0),
                stop=(j == NK - 1),
            )
        vt = vpool.tile([128, 512], BF16, tag="v")
        nc.scalar.copy(vt[:], ps[:])
        for h in range(HL):
            # One accumulation group for the whole bank: start clears the
            # entire PSUM bank, so only the very first matmul may set it.
            nc.tensor.matmul(
                pa[:, h * D:(h + 1) * D],
                sk_sb[t][:, h * D:(h + 1) * D],
                vt[:, h * D:(h + 1) * D],
                start=(t == 0 and h == 0),
                stop=(t == NT - 1 and h == HL - 1),
                skip_group_check=True,
            )

    # A_aug: per head [64, 65] = [A_h | ones]; stride-65 packing.
    # Rows 64-127 hold a copy so the ph3 lhsT/rhs base_partition can match
    # (qt rows 64-127 for odd local heads).
    a_aug = cpool.tile([128, HL * 65], BF16)
    nc.gpsimd.memset(
        a_aug[0:64, :].rearrange("p (h c) -> p h c", c=65)[:, :, 64:65], 1.0
    )
    nc.vector.tensor_copy(
        a_aug[0:64, :].rearrange("p (h c) -> p h c", c=65)[:, :, 0:64],
        pa[:].rearrange("p (h d) -> p h d", d=D),
    )
    nc.sync.dma_start(out=a_aug[64:128, :], in_=a_aug[0:64, :])

    # ---------------- phase 3: Q -> expQ^T -> Bt^T -> W_O ----------------
    xq_sb = load_x(xpool, 0, "x")
    wq_sb = load_w(wpool, 0, 512, "w")

    for fc in range(4):
        qt = qpool.tile([128, S], BF16, tag="qT")
        for sc in range(4):
            ps = ppool.tile([128, 512], F32, tag="pbig")
            for j in range(NK):
                k = (sc + j) % NK
                nc.tensor.matmul(
                    ps[:],
                    wq_sb[k][:, fc * 128:(fc + 1) * 128],
                    xq_sb[k][:, sc * 512:(sc + 1) * 512],
                    start=(j == 0),
                    stop=(j == NK - 1),
                )
            nc.scalar.activation(qt[:, sc * 512:(sc + 1) * 512], ps[:], AF.Exp)

        for hh in range(2):
            h = 2 * fc + hh       # local head
            btT = btpool.tile([128, S], BF16, tag="btT")
            for sc in range(4):
                # [65, 512]: rows 0-63 = unnormalized Bt^T chunk,
                # row 64 = qsum (ones column of A_aug)
                p6 = p6pool.tile([65, 512], F32, tag="p6")
                nc.tensor.matmul(
                    p6[:],
                    a_aug[hh * 64:(hh + 1) * 64, h * 65:(h + 1) * 65],
                    qt[hh * 64:(hh + 1) * 64, sc * 512:(sc + 1) * 512],
                    start=True,
                    stop=True,
                )
                qsr = qsrpool.tile([1, 512], F32, tag="qsr")
                nc.scalar.copy(qsr[:], p6[64:65, :])
                # rank-1 matmul broadcasts the qsum row to 64 partitions
                pb = pbpool.tile([64, 512], F32, tag="pb")
                nc.tensor.matmul(pb[:], ones_col[:], qsr[:],
                                 start=True, stop=True)
                rec = recpool.tile([64, 512], F32, tag="rec")
                nc.vector.reciprocal(rec[:], pb[:])
                nc.vector.tensor_mul(
                    btT[0:64, sc * 512:(sc + 1) * 512], p6[0:64, :], rec[:]
                )
            # rows 64-127 = rows 0-63 shifted left one column: lhsT chunk c
            # for W_O is then the single strided view btTv[:, 2c, :] with
            # partition p = tokl*64 + d -> element BtT[d, 16r + 2c + tokl].
            # (col S-1 of rows 64-127 stays unwritten; never read.)
            nc.sync.dma_start(out=btT[64:128, 0:S - 1], in_=btT[0:64, 1:S])
            btTv = btT[:].rearrange("p (r st) -> p st r", st=16)
            for oh in range(2):
                po = ppool.tile([128, 512], F32, tag="pbig")
                for c in range(NK):
                    nc.tensor.matmul(
                        po[:],
                        btTv[:, 2 * c, :],
                        wo_sb[c][:, oh * 512:(oh + 1) * 512],
                        start=(c == 0),
                        stop=(c == NK - 1),
                    )
                ob = opool.tile([128, 512], BF16, tag="osb")
                nc.scalar.copy(ob[:], po[:])
                nc.sync.dma_start(
                    out=out_ext[h * 128:(h + 1) * 128, oh * 512:(oh + 1) * 512],
                    in_=ob[:],
                )


_NC_CACHE = None


def _build():
    global _NC_CACHE
    if _NC_CACHE is not None:
        return _NC_CACHE
    nc = bacc_mod.Bacc(None, target_bir_lowering=False)
    xall = nc.declare_dram_parameter("xall", [3 * M, S], BF16, isOutput=False)
    wall = nc.declare_dram_parameter("wall", [M, 2560], BF16, isOutput=False)
    out = nc.declare_dram_parameter("out", [HL * 128, M], BF16, isOutput=True)
    with tile.TileContext(nc) as tc, ExitStack() as ctx:
        _emit(ctx, tc, nc, xall, wall, out)
    if not nc.is_finalized():
        nc.finalize()
    _NC_CACHE = nc
    return nc


def _in_maps(x_q, x_k, x_v, W_Q, W_K, W_V, W_O):
    xalls = []
    for b in range(4):
        xa = np.empty((3 * M, S), dtype=NP_BF16)
        # cast to bf16 while still contiguous (fast), then transpose the
        # 2-byte elements — ~2x faster than a strided f32->bf16 assign
        xa[0:M] = x_q[b].astype(NP_BF16).T
        xa[M:2 * M] = x_k[b].astype(NP_BF16).T
        xa[2 * M:3 * M] = x_v[b].astype(NP_BF16).T
        xalls.append(xa)
    walls = []
    for g in range(2):
        sl = slice(g * HL, (g + 1) * HL)
        wa = np.empty((M, 2560), dtype=NP_BF16)
        wa[:, 0:512] = (W_Q[sl] / D_SCALE).transpose(1, 0, 2).reshape(M, 512)
        wa[:, 512:1024] = (W_K[sl] / D_SCALE).transpose(1, 0, 2).reshape(M, 512)
        wa[:, 1024:1536] = W_V[sl].transpose(1, 0, 2).reshape(M, 512)
        wa[:, 1536:2560] = W_O.T
        walls.append(wa)
    return [{"xall": xalls[b], "wall": walls[g]}
            for b in range(4) for g in range(2)]


def run(inputs, **kw):
    nc = _build()
    maps = _in_maps(inputs["x_q"], inputs["x_k"], inputs["x_v"],
                    inputs["W_Q"], inputs["W_K"], inputs["W_V"],
                    inputs["W_O"])
    res = run_bass_kernel_spmd(nc, maps, list(range(8)), **kw)
    out = np.empty((4, S, M), dtype=np.float32)
    for b in range(4):
        for g in range(2):
            out[b, g * M:(g + 1) * M, :] = res.results[b * 2 + g]["out"]
    return out, res


def kernel(**inputs):
    out, _ = run(inputs)
    return out


def measure_exec_ns(inputs):
    """Neuron-profile NEFF execution time (max over the 8 cores), in ns.

    The axon client image lacks ``antenv.axon_hooks``, so
    ``run_bass_kernel_spmd(trace=True)`` cannot register the NTFF hook
    itself; drive the same capture directly: the ctypes profile hook from
    ``trn_agent_boot`` around the same bass2jax execute path that
    ``run_bass_kernel_spmd`` uses, then gauge's NTFF -> perfetto pipeline
    to extract per-core exec_time_ns.  Returns (exec_ns, source) where
    source is "ntff" or "staged-execute", or (None, None).
    """
    import glob as _glob
    import os as _os
    import sys as _sys
    import time as _time
    try:
        _sys.path.insert(0, "/root/.axon_site")
        from trn_agent_boot.trn_boot import _ntff_profile_via_ctypes
        from concourse import bass2jax
        from concourse._compat import FishPath
        import gauge.profiler
        import tempfile

        nc = _build()
        maps = _in_maps(inputs["x_q"], inputs["x_k"], inputs["x_v"],
                        inputs["W_Q"], inputs["W_K"], inputs["W_V"],
                        inputs["W_O"])
        hook = _ntff_profile_via_ctypes('/opt/axon/libaxon_pjrt.so')
        if hook is None:
            raise RuntimeError("no profile symbols in libaxon_pjrt.so")
        neff_dir = tempfile.mkdtemp(prefix="ntff_")
        # warm run so NEFF load / first-exec effects are not profiled
        bass2jax.run_bass_via_pjrt(nc, maps, n_cores=8)
        with hook(neff_dir, list(range(8))):
            bass2jax.run_bass_via_pjrt(nc, maps, n_cores=8)
        if not _glob.glob(_os.path.join(neff_dir, "*_body*.ntff")):
            raise RuntimeError("no NTFF produced")
        profile = gauge.profiler.Profile(
            profile_path=FishPath(neff_dir),
            kernel_dev_mode=True,
            profile_on_exit=False,
            bass_kernel=nc.m,
            offline_processing=True,
            fname="*_body*",
        )
        results = profile.to_perfetto(model_index=tuple(range(8)))
        times = [r.exec_time_ns for r in results if r.exec_time_ns]
        if times:
            return max(times), "ntff"
    except Exception:
        pass
    try:
        return _measure_staged_execute_ns(inputs, iters=5), "staged-execute"
    except Exception:
        return None, None


def _measure_staged_execute_ns(inputs, iters=5):
    """Fallback bound: blocking execute with inputs staged on-device
    (includes the ~67ms axon dispatch round-trip on top of silicon time)."""
    import time
    from jax.sharding import Mesh, PartitionSpec, NamedSharding
    from jax.experimental.shard_map import shard_map
    from concourse.bass2jax import (
        _bass_exec_p, partition_id_tensor, install_neuronx_cc_hook,
    )

    nc = _build()
    maps = _in_maps(inputs["x_q"], inputs["x_k"], inputs["x_v"],
                    inputs["W_Q"], inputs["W_K"], inputs["W_V"],
                    inputs["W_O"])
    install_neuronx_cc_hook()

    partition_name = (nc.partition_id_tensor.name
                      if nc.partition_id_tensor else None)
    in_names, out_names, out_avals, zero_outs = [], [], [], []
    for alloc in nc.m.functions[0].allocations:
        if not isinstance(alloc, mybir.MemoryLocationSet):
            continue
        name = alloc.memorylocations[0].name
        if alloc.kind == "ExternalInput":
            if name != partition_name:
                in_names.append(name)
        elif alloc.kind == "ExternalOutput":
            out_names.append(name)
            shape = tuple(alloc.tensor_shape)
            dtype = mybir.dt.np(alloc.dtype)
            out_avals.append(jax.core.ShapedArray(shape, dtype))
            zero_outs.append(np.zeros(shape, dtype))
    n_params = len(in_names)
    n_outs = len(out_avals)
    in_names.extend(out_names)
    if partition_name is not None:
        in_names.append(partition_name)
    donate = tuple(range(n_params, n_params + n_outs))

    def _body(*args):
        operands = list(args)
        if partition_name is not None:
            operands.append(partition_id_tensor())
        outs = _bass_exec_p.bind(
            *operands, out_avals=tuple(out_avals),
            in_names=tuple(in_names), out_names=tuple(out_names),
            lowering_input_output_aliases=(), sim_require_finite=True,
            sim_require_nnan=True, nc=nc)
        return tuple(outs)

    n_cores = 8
    devices = jax.devices()[:n_cores]
    mesh = Mesh(np.asarray(devices), ("core",))
    in_specs = (PartitionSpec("core"),) * (n_params + n_outs)
    out_specs = (PartitionSpec("core"),) * len(out_names)
    sharded = jax.jit(
        shard_map(_body, mesh=mesh, in_specs=in_specs,
                  out_specs=out_specs, check_rep=False),
        donate_argnums=donate, keep_unused=True)

    per_core = [[np.asarray(m[name]) for name in in_names[:n_params]]
                for m in maps]
    concat_in = [
        np.concatenate([per_core[c][i] for c in range(n_cores)], axis=0)
        for i in range(n_params)
    ]
    concat_zeros = [
        np.zeros((n_cores * z.shape[0], *z.shape[1:]), z.dtype)
        for z in zero_outs
    ]
    compiled = sharded.lower(*concat_in, *concat_zeros).compile()
    sharding = NamedSharding(mesh, PartitionSpec("core"))
    dev_in = [jax.device_put(a, sharding) for a in concat_in]
    for d in dev_in:
        d.block_until_ready()

    best = None
    for _ in range(iters):
        dev_zero = [jax.device_put(a, sharding) for a in concat_zeros]
        for d in dev_zero:
            d.block_until_ready()
        t0 = time.time()
        outs = compiled(*dev_in, *dev_zero)
        for o in outs:
            o.block_until_ready()
        dt = time.time() - t0
        if best is None or dt < best:
            best = dt
    return int(best * 1e9)


# revision 19
# speedup vs baseline: 15287.9017x; 1.0653x over previous
"""MHLA2 Trainium2 kernel — 8-core SPMD (batch x head-group sharding).

Math (per batch b, head h):
  Q=x_q@W_Q[h], K=x_k@W_K[h], V=x_v@W_V[h]          [S, 64]
  SK = softmax(K/ds) over d (row-wise)               [S, 64]
  A  = SK^T @ V                                      [64, 64]
  Bt = softmax(Q/ds) @ A                             [S, 64]
  torch-view reshape [b,h,s,d]->[b,s',f]: head h owns output rows
  s' in [h*128,(h+1)*128); Btr_h = Bt_h.reshape(128, 1024)
  out rows = Btr_h @ W_O^T                           [128, 1024]

Sharding: core c = b*2 + g handles batch b, heads g*8..g*8+7 and writes
the contiguous output block out[b, g*1024:(g+1)*1024, :].

All DRAM traffic is bf16 (tolerance 2e-2; measured ~4e-3): inputs are
packed into two bf16 params per core (xall = [xqT; xkT; xvT], wall =
[wq | wk | wv | woT]) and the output block is bf16.  Inputs are
uploaded directly (no collectives): NTFF profiling showed the
AllGather path costs ~300us of serial CC-stream time (init barrier
56us + ~45us fixed per gather) against a ~250us compute span, so
on-silicon exec time is lower with plain duplicated uploads.

On-chip pipeline per core (S=2048, M=1024, 8 local heads):
  ph1: K-proj (xkT resident, rotated k-accum) -> exp -> rowsum ->
       normalize (f32) -> cast bf16
  ph2: V-proj per s-tile -> A accumulation (frees V tiles early) ->
       A_aug = per-head [A_h | ones] with rows 64-127 duplicated
  ph3: per f-chunk: Q-proj -> exp (unnormalized bf16) -> per head:
       BtT_unnorm = A_aug_h^T @ expQ^T in one N=512 matmul per s-chunk
       ([65, 512] PSUM: rows 0-63 = Bt^T, row 64 = qsum), qsum row
       broadcast to 64 partitions via a rank-1 matmul, reciprocal +
       multiply on DVE -> btT (bf16, [64, S]); rows 64-127 get a copy
       shifted left one column so the W_O lhsT [(tok,d), r] chunks are
       single strided views btTv[:, 2c, :]; W_O matmuls -> bf16 out.
       (This replaces a per-s-tile PE-transpose + normalize + parity
       copy chain that serialized DVE/ACT and let HAM re-throttle.)
"""

import numpy as np
import ml_dtypes
from contextlib import ExitStack

import jax

# Persist XLA-compiled executables across processes: run_bass_kernel_spmd
# re-jits a fresh closure every call, so without this each call pays
# ~0.25-0.45s re-compiling the identical wrapper module (the inner NEFF is
# already disk-cached by neuronxcc separately).
try:
    jax.config.update("jax_compilation_cache_dir", "/tmp/.jax_comp_cache")
    jax.config.update("jax_persistent_cache_min_entry_size_bytes", -1)
    jax.config.update("jax_persistent_cache_min_compile_time_secs", 0)
except Exception:
    pass

import concourse.bacc as bacc_mod
import concourse.mybir as mybir
import concourse.tile as tile
from concourse.bass_utils import run_bass_kernel_spmd

S = 2048
M = 1024
D = 64
HL = 8            # heads per core
NK = 8            # 128-row contraction chunks of d_model
NT = 16           # 128-token tiles of S
F32 = mybir.dt.float32
F32R = mybir.dt.float32r
BF16 = mybir.dt.bfloat16
NP_BF16 = ml_dtypes.bfloat16
AX = mybir.AxisListType
AF = mybir.ActivationFunctionType
D_SCALE = float(D) ** 0.25


def _emit(ctx, tc, nc, xall, wall, out_ext):
    xpool = ctx.enter_context(tc.tile_pool(name="x", bufs=9))
    wpool = ctx.enter_context(tc.tile_pool(name="w", bufs=8))
    wopool = ctx.enter_context(tc.tile_pool(name="wo", bufs=8))
    skpool = ctx.enter_context(tc.tile_pool(name="sk", bufs=16))
    skfpool = ctx.enter_context(tc.tile_pool(name="skf", bufs=2))
    vpool = ctx.enter_context(tc.tile_pool(name="v", bufs=3))
    qpool = ctx.enter_context(tc.tile_pool(name="qT", bufs=2))
    btpool = ctx.enter_context(tc.tile_pool(name="btT", bufs=4))
    spool = ctx.enter_context(tc.tile_pool(name="small", bufs=8))
    qsrpool = ctx.enter_context(tc.tile_pool(name="qsr", bufs=4))
    recpool = ctx.enter_context(tc.tile_pool(name="rec", bufs=4))
    opool = ctx.enter_context(tc.tile_pool(name="osb", bufs=2))
    cpool = ctx.enter_context(tc.tile_pool(name="const", bufs=2))
    ppool = ctx.enter_context(tc.tile_pool(name="pbig", bufs=4, space="PSUM"))
    papool = ctx.enter_context(tc.tile_pool(name="pa", bufs=1, space="PSUM"))
    p6pool = ctx.enter_context(tc.tile_pool(name="p6", bufs=3, space="PSUM"))

    def load_x(pool, row_base, tag):
        tiles = []
        for k in range(NK):
            t = pool.tile([128, S], BF16, tag=tag)
            r = row_base + k * 128
            nc.gpsimd.dma_start(out=t[:], in_=xall[r:r + 128, :])
            tiles.append(t)
        return tiles

    def load_w(pool, col_lo, width, tag):
        tiles = []
        for k in range(NK):
            t = pool.tile([128, width], BF16, tag=tag)
            nc.gpsimd.dma_start(
                out=t[:],
                in_=wall[k * 128:(k + 1) * 128, col_lo:col_lo + width],
            )
            tiles.append(t)
        return tiles

    # ---------------- phase 1: K projection + softmax ----------------
    xk_sb = load_x(xpool, M, "x")
    wk_sb = load_w(wpool, 512, 512, "w")

    sk_sb = []
    for t in range(NT):
        ps = ppool.tile([128, 512], F32, tag="pbig")
        for j in range(NK):
            k = (t + j) % NK
            nc.tensor.matmul(
                ps[:],
                xk_sb[k][:, t * 128:(t + 1) * 128],
                wk_sb[k][:],
                start=(j == 0),
                stop=(j == NK - 1),
            )
        skf = skfpool.tile([128, 512], F32, tag="skf")
        nc.scalar.activation(skf[:], ps[:], AF.Exp)
        ksum = spool.tile([128, 8], F32, tag="ksum")
        nc.vector.reduce_sum(
            ksum[:], skf[:].rearrange("p (h d) -> p h d", d=D), axis=AX.X
        )
        krec = spool.tile([128, 8], F32, tag="krec")
        nc.vector.reciprocal(krec[:], ksum[:])
        # one broadcast-AP multiply normalizes all 8 heads and casts to
        # bf16 in a single DVE op (krec column h scales head h's 64 cols)
        sk = skpool.tile([128, 512], BF16, tag="sk")
        nc.vector.tensor_mul(
            sk[:].rearrange("p (h d) -> p h d", d=D),
            skf[:].rearrange("p (h d) -> p h d", d=D),
            krec[:, :, None].broadcast_to((128, HL, D)),
        )
        sk_sb.append(sk)

    # ---------------- phase 2: V projection + A accumulation ----------------
    xv_sb = load_x(xpool, 2 * M, "x")
    wv_sb = load_w(wpool, 1024, 512, "w")

    pa = papool.tile([64, 512], F32, tag="pa")
    for t in range(NT):
        ps = ppool.tile([128, 512], F32, tag="pbig")
        for j in range(NK):
            k = (t + j) % NK
            nc.tensor.matmul(
                ps[:],
                xv_sb[k][:, t * 128:(t + 1) * 128],
                wv_sb[k][:],
                start=(j == 0),
                stop=(j == NK - 1),
            )
        vt = vpool.tile([128, 512], BF16, tag="v")
        nc.vector.tensor_copy(vt[:], ps[:])
        for h in range(HL):
            # One accumulation group for the whole bank: start clears the
            # entire PSUM bank, so only the very first matmul may set it.
            nc.tensor.matmul(
                pa[:, h * D:(h + 1) * D],
                sk_sb[t][:, h * D:(h + 1) * D],
                vt[:, h * D:(h + 1) * D],
                start=(t == 0 and h == 0),
                stop=(t == NT - 1 and h == HL - 1),
                skip_group_check=True,
            )

    # A_aug: per head [64, 65] = [A_h | ones]; stride-65 packing.
    # Rows 64-127 hold a copy so the ph3 lhsT/rhs base_partition can match
    # (qt rows 64-127 for odd local heads).
    a_aug = cpool.tile([128, HL * 65], BF16)
    nc.gpsimd.memset(
        a_aug[0:64, :].rearrange("p (h c) -> p h c", c=65)[:, :, 64:65], 1.0
    )
    nc.vector.tensor_copy(
        a_aug[0:64, :].rearrange("p (h c) -> p h c", c=65)[:, :, 0:64],
        pa[:].rearrange("p (h d) -> p h d", d=D),
    )
    nc.sync.dma_start(out=a_aug[64:128, :], in_=a_aug[0:64, :])

    # ---------------- phase 3: Q -> expQ^T -> Bt^T -> W_O ----------------
    xq_sb = load_x(xpool, 0, "x")
    wq_sb = load_w(wpool, 0, 512, "w")
    wo_sb = load_w(wopool, 1536, M, "wo")

    def emit_wo(h, btT):
        # W_O lhsT chunk c is the single strided view btTv[:, 2c, :]:
        # partition p = tokl*64 + d -> element BtT[d, 16r + 2c + tokl]
        # (rows 64-127 hold the one-column-left-shifted copy).
        btTv = btT[:].rearrange("p (r st) -> p st r", st=16)
        for oh in range(2):
            po = ppool.tile([128, 512], F32, tag="pbig")
            for c in range(NK):
                nc.tensor.matmul(
                    po[:],
                    btTv[:, 2 * c, :],
                    wo_sb[c][:, oh * 512:(oh + 1) * 512],
                    start=(c == 0),
                    stop=(c == NK - 1),
                )
            ob = opool.tile([128, 512], BF16, tag="osb")
            nc.scalar.copy(ob[:], po[:])
            nc.sync.dma_start(
                out=out_ext[h * 128:(h + 1) * 128, oh * 512:(oh + 1) * 512],
                in_=ob[:],
            )

    # Software-pipelined across fc: the W_O matmuls of f-chunk fc-1 keep
    # the PE busy (HAM warm) while fc's normalize chains (DVE recip ->
    # GpSimd partition-broadcast -> DVE mul) drain the p6 banks.
    pending = []          # [(head, btT), ...] from the previous fc
    for fc in range(4):
        qt = qpool.tile([128, S], BF16, tag="qT")
        for sc in range(4):
            ps = ppool.tile([128, 512], F32, tag="pbig")
            for j in range(NK):
                k = (sc + j) % NK
                nc.tensor.matmul(
                    ps[:],
                    wq_sb[k][:, fc * 128:(fc + 1) * 128],
                    xq_sb[k][:, sc * 512:(sc + 1) * 512],
                    start=(j == 0),
                    stop=(j == NK - 1),
                )
            nc.scalar.activation(qt[:, sc * 512:(sc + 1) * 512], ps[:], AF.Exp)

        fresh = []
        for hh in range(2):
            h = 2 * fc + hh       # local head
            btT = btpool.tile([128, S], BF16, tag="btT")
            for sc in range(4):
                # [65, 512]: rows 0-63 = unnormalized Bt^T chunk,
                # row 64 = qsum (ones column of A_aug)
                p6 = p6pool.tile([65, 512], F32, tag="p6")
                nc.tensor.matmul(
                    p6[:],
                    a_aug[hh * 64:(hh + 1) * 64, h * 65:(h + 1) * 65],
                    qt[hh * 64:(hh + 1) * 64, sc * 512:(sc + 1) * 512],
                    start=True,
                    stop=True,
                )
                qrec = qsrpool.tile([1, 512], F32, tag="qsr")
                nc.vector.reciprocal(qrec[:], p6[64:65, :])
                rec = recpool.tile([64, 512], F32, tag="rec")
                nc.gpsimd.partition_broadcast(rec[:], qrec[:])
                nc.vector.tensor_mul(
                    btT[0:64, sc * 512:(sc + 1) * 512], p6[0:64, :], rec[:]
                )
            # rows 64-127 = rows 0-63 shifted left one column (see emit_wo;
            # col S-1 of rows 64-127 stays unwritten; never read).
            nc.sync.dma_start(out=btT[64:128, 0:S - 1], in_=btT[0:64, 1:S])
            fresh.append((h, btT))
        for h, btT in pending:
            emit_wo(h, btT)
        pending = fresh
    for h, btT in pending:
        emit_wo(h, btT)


_NC_CACHE = None


def _build():
    global _NC_CACHE
    if _NC_CACHE is not None:
        return _NC_CACHE
    nc = bacc_mod.Bacc(None, target_bir_lowering=False)
    xall = nc.declare_dram_parameter("xall", [3 * M, S], BF16, isOutput=False)
    wall = nc.declare_dram_parameter("wall", [M, 2560], BF16, isOutput=False)
    out = nc.declare_dram_parameter("out", [HL * 128, M], BF16, isOutput=True)
    with tile.TileContext(nc) as tc, ExitStack() as ctx:
        _emit(ctx, tc, nc, xall, wall, out)
    if not nc.is_finalized():
        nc.finalize()
    _NC_CACHE = nc
    return nc


def _in_maps(x_q, x_k, x_v, W_Q, W_K, W_V, W_O):
    xalls = []
    for b in range(4):
        xa = np.empty((3 * M, S), dtype=NP_BF16)
        # cast to bf16 while still contiguous (fast), then transpose the
        # 2-byte elements — ~2x faster than a strided f32->bf16 assign
        xa[0:M] = x_q[b].astype(NP_BF16).T
        xa[M:2 * M] = x_k[b].astype(NP_BF16).T
        xa[2 * M:3 * M] = x_v[b].astype(NP_BF16).T
        xalls.append(xa)
    walls = []
    for g in range(2):
        sl = slice(g * HL, (g + 1) * HL)
        wa = np.empty((M, 2560), dtype=NP_BF16)
        wa[:, 0:512] = (W_Q[sl] / D_SCALE).transpose(1, 0, 2).reshape(M, 512)
        wa[:, 512:1024] = (W_K[sl] / D_SCALE).transpose(1, 0, 2).reshape(M, 512)
        wa[:, 1024:1536] = W_V[sl].transpose(1, 0, 2).reshape(M, 512)
        wa[:, 1536:2560] = W_O.T
        walls.append(wa)
    return [{"xall": xalls[b], "wall": walls[g]}
            for b in range(4) for g in range(2)]


def run(inputs, **kw):
    nc = _build()
    maps = _in_maps(inputs["x_q"], inputs["x_k"], inputs["x_v"],
                    inputs["W_Q"], inputs["W_K"], inputs["W_V"],
                    inputs["W_O"])
    res = run_bass_kernel_spmd(nc, maps, list(range(8)), **kw)
    out = np.empty((4, S, M), dtype=np.float32)
    for b in range(4):
        for g in range(2):
            out[b, g * M:(g + 1) * M, :] = res.results[b * 2 + g]["out"]
    return out, res


def kernel(**inputs):
    out, _ = run(inputs)
    return out


def measure_exec_ns(inputs):
    """Neuron-profile NEFF execution time (max over the 8 cores), in ns.

    The axon client image lacks ``antenv.axon_hooks``, so
    ``run_bass_kernel_spmd(trace=True)`` cannot register the NTFF hook
    itself; drive the same capture directly: the ctypes profile hook from
    ``trn_agent_boot`` around the same bass2jax execute path that
    ``run_bass_kernel_spmd`` uses, then gauge's NTFF -> perfetto pipeline
    to extract per-core exec_time_ns.  Returns (exec_ns, source) where
    source is "ntff" or "staged-execute", or (None, None).
    """
    import glob as _glob
    import os as _os
    import sys as _sys
    import time as _time
    try:
        _sys.path.insert(0, "/root/.axon_site")
        from trn_agent_boot.trn_boot import _ntff_profile_via_ctypes
        from concourse import bass2jax
        from concourse._compat import FishPath
        import gauge.profiler
        import tempfile

        nc = _build()
        maps = _in_maps(inputs["x_q"], inputs["x_k"], inputs["x_v"],
                        inputs["W_Q"], inputs["W_K"], inputs["W_V"],
                        inputs["W_O"])
        hook = _ntff_profile_via_ctypes('/opt/axon/libaxon_pjrt.so')
        if hook is None:
            raise RuntimeError("no profile symbols in libaxon_pjrt.so")
        neff_dir = tempfile.mkdtemp(prefix="ntff_")
        # warm run so NEFF load / first-exec effects are not profiled
        bass2jax.run_bass_via_pjrt(nc, maps, n_cores=8)
        with hook(neff_dir, list(range(8))):
            bass2jax.run_bass_via_pjrt(nc, maps, n_cores=8)
        if not _glob.glob(_os.path.join(neff_dir, "*_body*.ntff")):
            raise RuntimeError("no NTFF produced")
        profile = gauge.profiler.Profile(
            profile_path=FishPath(neff_dir),
            kernel_dev_mode=True,
            profile_on_exit=False,
            bass_kernel=nc.m,
            offline_processing=True,
            fname="*_body*",
        )
        results = profile.to_perfetto(model_index=tuple(range(8)))
        times = [r.exec_time_ns for r in results if r.exec_time_ns]
        if times:
            return max(times), "ntff"
    except Exception:
        pass
    try:
        return _measure_staged_execute_ns(inputs, iters=5), "staged-execute"
    except Exception:
        return None, None


def _measure_staged_execute_ns(inputs, iters=5):
    """Fallback bound: blocking execute with inputs staged on-device
    (includes the ~67ms axon dispatch round-trip on top of silicon time)."""
    import time
    from jax.sharding import Mesh, PartitionSpec, NamedSharding
    from jax.experimental.shard_map import shard_map
    from concourse.bass2jax import (
        _bass_exec_p, partition_id_tensor, install_neuronx_cc_hook,
    )

    nc = _build()
    maps = _in_maps(inputs["x_q"], inputs["x_k"], inputs["x_v"],
                    inputs["W_Q"], inputs["W_K"], inputs["W_V"],
                    inputs["W_O"])
    install_neuronx_cc_hook()

    partition_name = (nc.partition_id_tensor.name
                      if nc.partition_id_tensor else None)
    in_names, out_names, out_avals, zero_outs = [], [], [], []
    for alloc in nc.m.functions[0].allocations:
        if not isinstance(alloc, mybir.MemoryLocationSet):
            continue
        name = alloc.memorylocations[0].name
        if alloc.kind == "ExternalInput":
            if name != partition_name:
                in_names.append(name)
        elif alloc.kind == "ExternalOutput":
            out_names.append(name)
            shape = tuple(alloc.tensor_shape)
            dtype = mybir.dt.np(alloc.dtype)
            out_avals.append(jax.core.ShapedArray(shape, dtype))
            zero_outs.append(np.zeros(shape, dtype))
    n_params = len(in_names)
    n_outs = len(out_avals)
    in_names.extend(out_names)
    if partition_name is not None:
        in_names.append(partition_name)
    donate = tuple(range(n_params, n_params + n_outs))

    def _body(*args):
        operands = list(args)
        if partition_name is not None:
            operands.append(partition_id_tensor())
        outs = _bass_exec_p.bind(
            *operands, out_avals=tuple(out_avals),
            in_names=tuple(in_names), out_names=tuple(out_names),
            lowering_input_output_aliases=(), sim_require_finite=True,
            sim_require_nnan=True, nc=nc)
        return tuple(outs)

    n_cores = 8
    devices = jax.devices()[:n_cores]
    mesh = Mesh(np.asarray(devices), ("core",))
    in_specs = (PartitionSpec("core"),) * (n_params + n_outs)
    out_specs = (PartitionSpec("core"),) * len(out_names)
    sharded = jax.jit(
        shard_map(_body, mesh=mesh, in_specs=in_specs,
                  out_specs=out_specs, check_rep=False),
        donate_argnums=donate, keep_unused=True)

    per_core = [[np.asarray(m[name]) for name in in_names[:n_params]]
                for m in maps]
    concat_in = [
        np.concatenate([per_core[c][i] for c in range(n_cores)], axis=0)
        for i in range(n_params)
    ]
    concat_zeros = [
        np.zeros((n_cores * z.shape[0], *z.shape[1:]), z.dtype)
        for z in zero_outs
    ]
    compiled = sharded.lower(*concat_in, *concat_zeros).compile()
    sharding = NamedSharding(mesh, PartitionSpec("core"))
    dev_in = [jax.device_put(a, sharding) for a in concat_in]
    for d in dev_in:
        d.block_until_ready()

    best = None
    for _ in range(iters):
        dev_zero = [jax.device_put(a, sharding) for a in concat_zeros]
        for d in dev_zero:
            d.block_until_ready()
        t0 = time.time()
        outs = compiled(*dev_in, *dev_zero)
        for o in outs:
            o.block_until_ready()
        dt = time.time() - t0
        if best is None or dt < best:
            best = dt
    return int(best * 1e9)
